# revision 1
# baseline (speedup 1.0000x reference)
"""CRD loss kernel for Trainium2, 8-core data-parallel SPMD.

loss = -sum_i( (zs_i . zt_i) / (|zs_i| |zt_i|) ) / B
  zs = f_s @ W_s.T + b_s   [B, 128]
  zt = f_t @ W_t.T + b_t   [B, 128]

Sharding: batch B=16384 split across 8 cores (2048 rows each); projection
weights replicated. Each core emits per-row-chunk partial sums [128, nblk];
the host sums all of them and scales.

Per-core dataflow (fp32 data, fp32r matmul arithmetic, ~3e-4 rel err):
  - x tiles [128, D] DMA'd naturally (rows on partitions); memory-bound
    stream of 14.7 MB/core is the roofline.
  - PE transposes 128x128 blocks into PSUM; DVE/ACT copy them to SBUF as
    fp32r -> xT tiles [dim-chunk 128, rows<=512].
  - z.T [feat 128, rows] = sum_k (W.T chunk).T @ xT chunk accumulated in
    PSUM; bias added via a rank-1 (b x ones_row) accumulating matmul.
  - zs.T/zt.T copied to SBUF (fp32r); zs*zt (DVE), zs^2 (ACT), zt^2 (DVE).
  - Row sums land ON PARTITIONS via matmul(lhsT=product chunk, rhs=ones
    [128,1]) -> [rows128, 1] columns of one PSUM tile, so the normalize
    tail (reciprocal, sqrt, muls, reduce) is partition-parallel.
  - Tapered final blocks (512,512,512,256,256) shorten the critical chain
    behind the last DMA.
"""
import numpy as np

import concourse.bass as bass
import concourse.mybir as mybir
from concourse.tile import TileContext
from concourse import bass_utils
from concourse.masks import make_identity

# Problem shapes (hardcoded per contest contract)
B = 16384
DS = 768
DT = 1024
F = 128
NCORES = 8
R = B // NCORES          # rows per core = 2048
BLK = 512                # max row block (fp32 moving-operand max)
# (row_offset, rows): tapered final blocks shorten the post-last-DMA chain
BLOCKS = [(0, 512), (512, 512), (1024, 512), (1536, 256), (1792, 256)]
NBLK = len(BLOCKS)
P = 128

f32 = mybir.dt.float32
f32r = mybir.dt.float32r

_CACHE = {}


def legalize_waits(nc, max_waits=1):
    """Walrus codegen in this container rejects >1 sync-wait per instruction.
    Split extra waits onto same-engine NoOps placed right before the instr."""
    n_fixed = 0
    for fn in nc.m.functions:
        for blk in fn.blocks:
            new_insts = []
            for inst in blk.instructions:
                si = inst.sync_info
                if (
                    si is not None
                    and len(si.on_wait) > max_waits
                    and not isinstance(inst, mybir.InstISA)
                ):
                    waits = list(si.on_wait)
                    extra, keep = waits[:-max_waits], waits[-max_waits:]
                    for j, w in enumerate(extra):
                        nop = mybir.InstNoOp(
                            name=f"{inst.name}-wn{j}", engine=inst.engine
                        )
                        nop.sync_info = mybir.SyncInfo(on_wait=[w], on_update=[])
                        new_insts.append(nop)
                    inst.sync_info = mybir.SyncInfo(
                        on_wait=keep, on_update=list(si.on_update)
                    )
                    n_fixed += 1
                new_insts.append(inst)
            blk.instructions = new_insts
    return n_fixed


def build(repeat=1):
    nc = bass.Bass("TRN2")
    fs = nc.dram_tensor("fs", [R, DS], f32, kind="ExternalInput")
    ft = nc.dram_tensor("ft", [R, DT], f32, kind="ExternalInput")
    wst = nc.dram_tensor("wst", [F, DS + DT], f32, kind="ExternalInput")
    bst = nc.dram_tensor("bst", [1, 2 * F], f32, kind="ExternalInput")
    out = nc.dram_tensor("out", [P, len(BLOCKS)], f32, kind="ExternalOutput")

    with TileContext(nc) as tc:
        with (
            tc.tile_pool(name="const", bufs=1) as const,
            tc.tile_pool(name="xnat_s", bufs=12) as xnat_s_pool,
            tc.tile_pool(name="xnat_t", bufs=12) as xnat_t_pool,
            tc.tile_pool(name="xT", bufs=8) as xT_pool,
            tc.tile_pool(name="zprod", bufs=4) as zprod_pool,
            tc.tile_pool(name="tail", bufs=2) as tail_pool,
            tc.tile_pool(name="psum_zs", bufs=1, space="PSUM") as psum_zs_pool,
            tc.tile_pool(name="psum_zt", bufs=1, space="PSUM") as psum_zt_pool,
            tc.tile_pool(name="psum_tp", bufs=5, space="PSUM") as psum_tp_pool,
            tc.tile_pool(name="psum_sum", bufs=1, space="PSUM") as psum_sum_pool,
        ):
            # ---- constants / weights prep ----
            identity = const.tile([P, P], f32)
            make_identity(nc, identity[:, :])
            identity_r = const.tile([P, P], f32r)
            nc.vector.tensor_copy(identity_r, identity)

            ones_col_f = const.tile([P, 1], f32)
            nc.vector.memset(ones_col_f, 1.0)
            ones_col = const.tile([P, 1], f32r)
            nc.vector.tensor_copy(ones_col, ones_col_f)

            ones_row_f = const.tile([1, BLK], f32)
            nc.vector.memset(ones_row_f, 1.0)
            ones_row = const.tile([1, BLK], f32r)
            nc.vector.tensor_copy(ones_row, ones_row_f)

            wst_nat = const.tile([F, DS + DT], f32)
            nc.sync.dma_start(wst_nat, wst[:, :])

            bst_nat = const.tile([1, 2 * F], f32)
            nc.sync.dma_start(bst_nat, bst[:, :])
            bst_r = const.tile([1, 2 * F], f32r)
            nc.vector.tensor_copy(bst_r, bst_nat)
            bs_r = bst_r[0:1, 0:F]
            bt_r = bst_r[0:1, F:2 * F]

            # W.T chunks, fp32r: wT[:, k*128:(k+1)*128] = W[:, chunk k].T
            wsT = const.tile([P, DS], f32r)
            wtT = const.tile([P, DT], f32r)
            for w_nat, w_T, D in (
                (wst_nat[:, 0:DS], wsT, DS),
                (wst_nat[:, DS:DS + DT], wtT, DT),
            ):
                nch = D // P
                for k0 in range(0, nch, 4):
                    kw = min(4, nch - k0)
                    tp = psum_tp_pool.tile([P, BLK], f32, tag="tp")
                    for j in range(kw):
                        k = k0 + j
                        nc.tensor.transpose(
                            tp[:, j * P:(j + 1) * P],
                            w_nat[:, k * P:(k + 1) * P],
                            identity,
                        )
                    nc.vector.tensor_copy(
                        w_T[:, k0 * P:(k0 + kw) * P], tp[:, : kw * P]
                    )

            partials = const.tile([P, len(BLOCKS)], f32)

            # ---- main loop over row blocks ----
            branch_cfg = {
                "s": (fs, DS, wsT, bs_r, xnat_s_pool),
                "t": (ft, DT, wtT, bt_r, xnat_t_pool),
            }
            for blk, (r0blk, rows) in [
                bl for _ in range(repeat) for bl in enumerate(BLOCKS)
            ]:
                nrt = rows // P
                psum_z = {}
                # last block: t first so the final post-DMA chain is the
                # shorter s branch
                order = ("s", "t") if blk < len(BLOCKS) - 1 else ("t", "s")
                for br in order:
                    x_dram, D, w_T, b_r, xpool = branch_cfg[br]
                    nch = D // P
                    # SWDGE cast-DMAs (f32 -> f32r rounding in the DMA), one
                    # per 128-row tile for fine-grained transpose overlap.
                    # Final block's trailing branch: column-split halves so
                    # early k-chunks' transposes start before the tile
                    # finishes loading.
                    split_cols = blk == len(BLOCKS) - 1 and br == order[-1]
                    x_tiles = []
                    for rt in range(nrt):
                        xn = xpool.tile([P, D], f32r, tag=f"xn_{br}")
                        r0 = r0blk + rt * P
                        if split_cols:
                            h = D // 2
                            nc.gpsimd.dma_start(
                                xn[:, 0:h], x_dram[r0:r0 + P, 0:h]
                            )
                            nc.gpsimd.dma_start(
                                xn[:, h:D], x_dram[r0:r0 + P, h:D]
                            )
                        else:
                            nc.gpsimd.dma_start(xn, x_dram[r0:r0 + P, :])
                        x_tiles.append(xn)

                    psz = (psum_zs_pool if br == "s" else psum_zt_pool).tile(
                        [P, rows], f32
                    )
                    psum_z[br] = psz
                    for k in range(nch):
                        tp = psum_tp_pool.tile([P, rows], f32r, tag="tp")
                        for rt in range(nrt):
                            nc.tensor.transpose(
                                tp[:, rt * P:(rt + 1) * P],
                                x_tiles[rt][:, k * P:(k + 1) * P],
                                identity_r,
                            )
                        xT = xT_pool.tile([P, rows], f32r, tag="xT")
                        if k % 2 == 0:
                            nc.vector.tensor_copy(xT, tp)
                        else:
                            nc.scalar.copy(xT, tp)
                        nc.tensor.matmul(
                            psz,
                            w_T[:, k * P:(k + 1) * P],
                            xT,
                            start=(k == 0),
                            stop=False,
                        )
                    # bias: rank-1 update b (x) ones_row
                    nc.tensor.matmul(
                        psz, b_r, ones_row[:, :rows], start=False, stop=True
                    )

                # products and squares (fp32r SBUF)
                zs_sb = zprod_pool.tile([P, rows], f32r, tag="zsb")
                zt_sb = zprod_pool.tile([P, rows], f32r, tag="zsb")
                nc.vector.tensor_copy(zs_sb, psum_z["s"])
                nc.scalar.copy(zt_sb, psum_z["t"])

                prod_st = zprod_pool.tile([P, rows], f32, tag="prod")
                zs2 = zprod_pool.tile([P, rows], f32, tag="prod")
                zt2 = zprod_pool.tile([P, rows], f32, tag="prod")
                nc.vector.tensor_mul(prod_st, zs_sb, zt_sb)
                nc.scalar.square(zs2, zs_sb)
                nc.vector.tensor_mul(zt2, zt_sb, zt_sb)

                # row sums on PARTITIONS: matmul(lhsT=prod chunk [feat, rows128],
                # rhs=ones [feat,1]) -> [rows128, 1]. Columns of sumsT:
                # c + nchunks*{0: st, 1: ss, 2: tt} for row chunk c.
                nchunks = rows // P
                sumsT = psum_sum_pool.tile([P, 3 * nchunks], f32, tag="sumsT")
                for i, src in enumerate((prod_st, zs2, zt2)):
                    for c in range(nchunks):
                        nc.tensor.matmul(
                            sumsT[:, i * nchunks + c:i * nchunks + c + 1],
                            src[:, c * P:(c + 1) * P],
                            ones_col_f,
                            start=True,
                            stop=True,
                        )
                sumsT_sb = tail_pool.tile([P, 3 * nchunks], f32, tag="sumsT")
                nc.vector.tensor_copy(sumsT_sb, sumsT)

                # tail (all [128, nchunks]-shaped, partition-parallel):
                # partial = sum st * rsqrt(ss) * rsqrt(tt)
                q = tail_pool.tile([P, 2 * nchunks], f32, tag="q")
                nc.vector.reciprocal(
                    q, sumsT_sb[:, nchunks:3 * nchunks]
                )
                q2 = tail_pool.tile([P, 2 * nchunks], f32, tag="q2")
                nc.scalar.activation(q2, q, mybir.ActivationFunctionType.Sqrt)
                v = tail_pool.tile([P, nchunks], f32, tag="v")
                nc.vector.tensor_mul(
                    v, q2[:, 0:nchunks], q2[:, nchunks:2 * nchunks]
                )
                w_ = tail_pool.tile([P, nchunks], f32, tag="w")
                nc.vector.tensor_mul(w_, sumsT_sb[:, 0:nchunks], v)
                nc.vector.reduce_sum(
                    partials[:, blk:blk + 1], w_, axis=mybir.AxisListType.X
                )

            # per-row-chunk partials [128, n_blocks]; host does the final sum
            nc.sync.dma_start(out[:, :], partials)

    legalize_waits(nc)
    return nc


def get_nc():
    if "nc" not in _CACHE:
        _CACHE["nc"] = build()
    return _CACHE["nc"]


def make_in_maps(f_s, f_t, W_s, b_s, W_t, b_t):
    f_s = np.ascontiguousarray(np.asarray(f_s, dtype=np.float32))
    f_t = np.ascontiguousarray(np.asarray(f_t, dtype=np.float32))
    W_s = np.ascontiguousarray(np.asarray(W_s, dtype=np.float32))
    b_s = np.ascontiguousarray(np.asarray(b_s, dtype=np.float32)).reshape(1, F)
    W_t = np.ascontiguousarray(np.asarray(W_t, dtype=np.float32))
    b_t = np.ascontiguousarray(np.asarray(b_t, dtype=np.float32)).reshape(1, F)
    wst = np.ascontiguousarray(np.concatenate([W_s, W_t], axis=1))
    bst = np.ascontiguousarray(np.concatenate([b_s, b_t], axis=1))
    in_maps = []
    for c in range(NCORES):
        sl = slice(c * R, (c + 1) * R)
        in_maps.append(
            {"fs": f_s[sl], "ft": f_t[sl], "wst": wst, "bst": bst}
        )
    return in_maps


def combine(results):
    total = sum(
        results[c]["out"].astype(np.float64).sum() for c in range(NCORES)
    )
    loss = -(total / B)
    return np.array([loss], dtype=np.float32)


def kernel(f_s, f_t, W_s, b_s, W_t, b_t):
    nc = get_nc()
    in_maps = make_in_maps(f_s, f_t, W_s, b_s, W_t, b_t)
    last_err = None
    for _ in range(3):  # retry transient device wedges (NRT_EXEC_UNIT_...)
        try:
            res = bass_utils.run_bass_kernel_spmd(
                nc, in_maps, core_ids=list(range(NCORES))
            )
            return combine(res.results)
        except Exception as e:  # noqa: BLE001
            last_err = e
    raise last_err



# revision 3
# speedup vs baseline: 2.0544x; 2.0544x over previous
"""CRD loss kernel for Trainium2, 8-core data-parallel SPMD.

loss = -sum_i( (zs_i . zt_i) / (|zs_i| |zt_i|) ) / B
  zs = f_s @ W_s.T + b_s   [B, 128]
  zt = f_t @ W_t.T + b_t   [B, 128]

Sharding: batch B=16384 split across 8 cores (2048 rows each); projection
weights replicated. Each core emits per-row-chunk partial sums [128, nblk];
the host sums all of them and scales.

Per-core dataflow (v2):
  - Host passes x TRANSPOSED ([D, rows], layout prep only) so no PE
    transposes are needed: z.T [feat, rows] = sum_k (W.T chunk).T @ xT chunk
    accumulated in PSUM directly from DMA'd xT tiles.
  - Three DMA queues run in parallel in the cost model (SP / ACT HWDGE and
    the Pool SWDGE queue), each ~332 GB/s effective. The Pool queue casts
    f32->f16 in the DMA (charged at OUTPUT bytes), so it carries half the
    k-chunks at 2x efficiency; SP and ACT split the rest as f32r.
    Mixed-precision accumulation: f16 chunks and f32r chunks accumulate into
    the same fp32 PSUM tile (c. 3e-4 final rel err).
  - Bias is fused into the PSUM->SBUF eviction: ACT Identity+bias(zs),
    DVE tensor_scalar add (zt); both emit f16 for 2x DVE products.
  - Row sums land ON PARTITIONS via matmul(lhsT=product chunk, rhs=ones
    [128,1]) -> [rows128, 1] columns of one PSUM tile, so the normalize
    tail (reciprocal, sqrt, muls, reduce) is partition-parallel.
  - Tapered final blocks (512,512,512,256,256) shorten the critical chain
    behind the last DMA.
"""
import numpy as np

import concourse.bass as bass
import concourse.mybir as mybir
from concourse.tile import TileContext
from concourse import bass_utils

# Problem shapes (hardcoded per contest contract)
B = 16384
DS = 768
DT = 1024
F = 128
NCORES = 8
R = B // NCORES          # rows per core = 2048
NCH_S = DS // 128        # 6
NCH_T = DT // 128        # 8
# (row_offset, rows): tapered first block fills the pipeline fast; tapered
# final blocks shorten the post-last-DMA chain
BLOCKS = [(0, 256), (256, 512), (768, 512), (1280, 512), (1792, 256)]
NBLK = len(BLOCKS)
P = 128

f32 = mybir.dt.float32
f32r = mybir.dt.float32r
f16 = mybir.dt.float16

# chunk -> stream assignment (fixed across blocks so stationary dtype is
# fixed per chunk). pool casts to f16; sp/act load f32r. t-chunk 7 rows are
# split between sp and act on 512-row blocks (sp-only on 256-row blocks) so
# each stream carries ~2.9us per 512 rows.
S_POOL = range(0, 3)     # s-chunks via pool (f16)
S_SP = range(3, 6)       # s-chunks via sp (f32r)
T_POOL = range(0, 4)     # t-chunks via pool (f16)
T_ACT = range(4, 7)      # t-chunks via act (f32r)
T_SPLIT = 7              # t-chunk 7: rows split between sp and act (f32r)

_CACHE = {}


def legalize_waits(nc, max_waits=1):
    """Walrus codegen in this container rejects >1 sync-wait per instruction.
    Split extra waits onto same-engine NoOps placed right before the instr."""
    n_fixed = 0
    for fn in nc.m.functions:
        for blk in fn.blocks:
            new_insts = []
            for inst in blk.instructions:
                si = inst.sync_info
                if (
                    si is not None
                    and len(si.on_wait) > max_waits
                    and not isinstance(inst, mybir.InstISA)
                ):
                    waits = list(si.on_wait)
                    extra, keep = waits[:-max_waits], waits[-max_waits:]
                    for j, w in enumerate(extra):
                        nop = mybir.InstNoOp(
                            name=f"{inst.name}-wn{j}", engine=inst.engine
                        )
                        nop.sync_info = mybir.SyncInfo(on_wait=[w], on_update=[])
                        new_insts.append(nop)
                    inst.sync_info = mybir.SyncInfo(
                        on_wait=keep, on_update=list(si.on_update)
                    )
                    n_fixed += 1
                new_insts.append(inst)
            blk.instructions = new_insts
    return n_fixed


def build(repeat=1):
    nc = bass.Bass("TRN2")
    # x transposed on host: [n_chunks, 128, R]; f32r == f32 bits
    fsT = nc.dram_tensor("fsT", [NCH_S, P, R], f32r, kind="ExternalInput")
    ftT = nc.dram_tensor("ftT", [NCH_T, P, R], f32r, kind="ExternalInput")
    # W in transposed-chunk layout: wst[p, k*128+f] = W[f, k*128+p]
    wst = nc.dram_tensor("wst", [P, DS + DT], f32r, kind="ExternalInput")
    bst = nc.dram_tensor("bst", [P, 2], f32, kind="ExternalInput")
    out = nc.dram_tensor("out", [P, NBLK], f32, kind="ExternalOutput")

    with TileContext(nc) as tc:
        with (
            tc.tile_pool(name="const", bufs=1) as const,
            tc.tile_pool(name="xs_pool", bufs=3) as xs_pool_p,
            tc.tile_pool(name="xs_sp", bufs=3) as xs_sp_p,
            tc.tile_pool(name="xt_pool", bufs=3) as xt_pool_p,
            tc.tile_pool(name="xt_act", bufs=3) as xt_act_p,
            tc.tile_pool(name="xt_spl", bufs=3) as xt_spl_p,
            tc.tile_pool(name="zprod", bufs=4) as zprod_pool,
            tc.tile_pool(name="tail", bufs=2) as tail_pool,
            tc.tile_pool(name="psum_zs", bufs=2, space="PSUM") as psum_zs_pool,
            tc.tile_pool(name="psum_zt", bufs=2, space="PSUM") as psum_zt_pool,
            tc.tile_pool(name="psum_sum", bufs=1, space="PSUM") as psum_sum_pool,
        ):
            # ---- constants / weights ----
            ones_col = const.tile([P, 1], f16)
            nc.vector.memset(ones_col, 1.0)

            # f16 stationary for pool-stream chunks (full W, one cast DMA)
            wst_f16 = const.tile([P, DS + DT], f16)
            nc.gpsimd.dma_start(wst_f16, wst[:, :])

            # f32r stationary only for the sp/act chunk ranges
            ws_f32r = const.tile([P, 3 * P], f32r)     # s-chunks 3..5
            nc.sync.dma_start(ws_f32r, wst[:, 3 * P:6 * P])
            wt_f32r = const.tile([P, 4 * P], f32r)     # t-chunks 4..7
            nc.scalar.dma_start(wt_f32r, wst[:, (NCH_S + 4) * P:(NCH_S + 8) * P])

            bst_sb = const.tile([P, 2], f32)
            nc.sync.dma_start(bst_sb, bst[:, :])
            bs_col = bst_sb[:, 0:1]
            bt_col = bst_sb[:, 1:2]

            partials = const.tile([P, NBLK], f32)

            def w16(kglob):
                return wst_f16[:, kglob * P:(kglob + 1) * P]

            # ---- main loop over row blocks ----
            for blk, (r0, rows) in [
                bl for _ in range(repeat) for bl in enumerate(BLOCKS)
            ]:
                last = blk == NBLK - 1
                # x tile loads: three parallel queues
                xs_po = xs_pool_p.tile([P, len(S_POOL), rows], f16, tag="xs_po")
                nc.gpsimd.dma_start(
                    xs_po,
                    fsT[S_POOL.start:S_POOL.stop, :, r0:r0 + rows].rearrange(
                        "k p r -> p k r"
                    ),
                )
                xt_po = xt_pool_p.tile([P, len(T_POOL), rows], f16, tag="xt_po")
                nc.gpsimd.dma_start(
                    xt_po,
                    ftT[T_POOL.start:T_POOL.stop, :, r0:r0 + rows].rearrange(
                        "k p r -> p k r"
                    ),
                )
                xs_sp = xs_sp_p.tile([P, len(S_SP), rows], f32r, tag="xs_sp")
                nc.sync.dma_start(
                    xs_sp,
                    fsT[S_SP.start:S_SP.stop, :, r0:r0 + rows].rearrange(
                        "k p r -> p k r"
                    ),
                )
                xt_ac = xt_act_p.tile([P, len(T_ACT), rows], f32r, tag="xt_ac")
                nc.scalar.dma_start(
                    xt_ac,
                    ftT[T_ACT.start:T_ACT.stop, :, r0:r0 + rows].rearrange(
                        "k p r -> p k r"
                    ),
                )
                # t-chunk 7: rows split between the two HWDGE queues
                xt_sl = xt_spl_p.tile([P, rows], f32r, tag="xt_sl")
                h = rows // 2
                nc.sync.dma_start(xt_sl[:, 0:h], ftT[T_SPLIT, :, r0:r0 + h])
                nc.scalar.dma_start(
                    xt_sl[:, h:rows], ftT[T_SPLIT, :, r0 + h:r0 + rows]
                )

                # ---- projections: z.T [feat, rows] accumulated in PSUM ----
                psz_s = psum_zs_pool.tile([P, rows], f32, tag="psz_s")
                psz_t = psum_zt_pool.tile([P, rows], f32, tag="psz_t")

                def mm_s():
                    n = len(S_POOL) + len(S_SP)
                    i = 0
                    for k in S_POOL:
                        nc.tensor.matmul(
                            psz_s, w16(k), xs_po[:, k - S_POOL.start, :],
                            start=(i == 0), stop=(i == n - 1),
                        )
                        i += 1
                    for k in S_SP:
                        nc.tensor.matmul(
                            psz_s,
                            ws_f32r[:, (k - 3) * P:(k - 2) * P],
                            xs_sp[:, k - S_SP.start, :],
                            start=(i == 0), stop=(i == n - 1),
                        )
                        i += 1

                def mm_t():
                    n = len(T_POOL) + len(T_ACT) + 1
                    i = 0
                    for k in T_POOL:
                        nc.tensor.matmul(
                            psz_t, w16(NCH_S + k), xt_po[:, k - T_POOL.start, :],
                            start=(i == 0), stop=(i == n - 1),
                        )
                        i += 1
                    for k in T_ACT:
                        nc.tensor.matmul(
                            psz_t,
                            wt_f32r[:, (k - 4) * P:(k - 3) * P],
                            xt_ac[:, k - T_ACT.start, :],
                            start=(i == 0), stop=(i == n - 1),
                        )
                        i += 1
                    nc.tensor.matmul(
                        psz_t, wt_f32r[:, 3 * P:4 * P], xt_sl,
                        start=False, stop=True,
                    )

                # last block: t first so the final post-DMA chain is the
                # shorter s branch
                if last:
                    mm_t(); mm_s()
                else:
                    mm_s(); mm_t()

                # ---- bias + eviction to f16 SBUF ----
                zs_sb = zprod_pool.tile([P, rows], f16, tag="zsb")
                zt_sb = zprod_pool.tile([P, rows], f16, tag="ztb")
                nc.scalar.activation(
                    zs_sb, psz_s, mybir.ActivationFunctionType.Identity,
                    bias=bs_col,
                )
                nc.vector.tensor_scalar(
                    zt_sb, psz_t, bt_col, None, mybir.AluOpType.add
                )

                # products and squares (f16)
                prod_st = zprod_pool.tile([P, rows], f16, tag="prod")
                zs2 = zprod_pool.tile([P, rows], f16, tag="zs2")
                zt2 = zprod_pool.tile([P, rows], f16, tag="zt2")
                nc.vector.tensor_mul(prod_st, zs_sb, zt_sb)
                nc.scalar.square(zs2, zs_sb)
                nc.vector.tensor_mul(zt2, zt_sb, zt_sb)

                # row sums on PARTITIONS: matmul(lhsT=prod chunk [feat, rows128],
                # rhs=ones [feat,1]) -> [rows128, 1]. Columns of sumsT:
                # c + nchunks*{0: st, 1: ss, 2: tt} for row chunk c.
                nchunks = rows // P
                sumsT = psum_sum_pool.tile([P, 3 * nchunks], f32, tag="sumsT")
                for i, src in enumerate((prod_st, zs2, zt2)):
                    for c in range(nchunks):
                        nc.tensor.matmul(
                            sumsT[:, i * nchunks + c:i * nchunks + c + 1],
                            src[:, c * P:(c + 1) * P],
                            ones_col,
                            start=True,
                            stop=True,
                        )
                sumsT_sb = tail_pool.tile([P, 3 * nchunks], f32, tag="sumsT")
                nc.vector.tensor_copy(sumsT_sb, sumsT)

                # tail (all [128, nchunks]-shaped, partition-parallel):
                # partial = sum st * rsqrt(ss) * rsqrt(tt)
                q = tail_pool.tile([P, 2 * nchunks], f32, tag="q")
                nc.vector.reciprocal(
                    q, sumsT_sb[:, nchunks:3 * nchunks]
                )
                q2 = tail_pool.tile([P, 2 * nchunks], f32, tag="q2")
                nc.scalar.activation(q2, q, mybir.ActivationFunctionType.Sqrt)
                v = tail_pool.tile([P, nchunks], f32, tag="v")
                nc.vector.tensor_mul(
                    v, q2[:, 0:nchunks], q2[:, nchunks:2 * nchunks]
                )
                w_ = tail_pool.tile([P, nchunks], f32, tag="w")
                nc.vector.tensor_mul(w_, sumsT_sb[:, 0:nchunks], v)
                nc.vector.reduce_sum(
                    partials[:, blk:blk + 1], w_, axis=mybir.AxisListType.X
                )

            # per-row-chunk partials [128, n_blocks]; host does the final sum
            nc.sync.dma_start(out[:, :], partials)

    legalize_waits(nc)
    return nc


def get_nc():
    if "nc" not in _CACHE:
        _CACHE["nc"] = build()
    return _CACHE["nc"]


def make_in_maps(f_s, f_t, W_s, b_s, W_t, b_t):
    f_s = np.ascontiguousarray(np.asarray(f_s, dtype=np.float32))
    f_t = np.ascontiguousarray(np.asarray(f_t, dtype=np.float32))
    W_s = np.asarray(W_s, dtype=np.float32)
    b_s = np.asarray(b_s, dtype=np.float32)
    W_t = np.asarray(W_t, dtype=np.float32)
    b_t = np.asarray(b_t, dtype=np.float32)

    # layout prep (no arithmetic): x transposed, W in transposed-chunk form
    fsT = np.ascontiguousarray(f_s.T)    # [DS, B]
    ftT = np.ascontiguousarray(f_t.T)    # [DT, B]
    wst = np.empty((P, DS + DT), dtype=np.float32)
    for k in range(NCH_S):
        wst[:, k * P:(k + 1) * P] = W_s[:, k * P:(k + 1) * P].T
    for k in range(NCH_T):
        wst[:, (NCH_S + k) * P:(NCH_S + k + 1) * P] = W_t[:, k * P:(k + 1) * P].T
    bst = np.ascontiguousarray(
        np.stack([b_s, b_t], axis=1)
    )  # [128, 2]

    in_maps = []
    for c in range(NCORES):
        sl = slice(c * R, (c + 1) * R)
        in_maps.append(
            {
                "fsT": np.ascontiguousarray(fsT[:, sl]).reshape(NCH_S, P, R),
                "ftT": np.ascontiguousarray(ftT[:, sl]).reshape(NCH_T, P, R),
                "wst": wst,
                "bst": bst,
            }
        )
    return in_maps


def combine(results):
    total = sum(
        results[c]["out"].astype(np.float64).sum() for c in range(NCORES)
    )
    loss = -(total / B)
    return np.array([loss], dtype=np.float32)


def kernel(f_s, f_t, W_s, b_s, W_t, b_t):
    nc = get_nc()
    in_maps = make_in_maps(f_s, f_t, W_s, b_s, W_t, b_t)
    last_err = None
    for _ in range(3):  # retry transient device wedges (NRT_EXEC_UNIT_...)
        try:
            res = bass_utils.run_bass_kernel_spmd(
                nc, in_maps, core_ids=list(range(NCORES))
            )
            return combine(res.results)
        except Exception as e:  # noqa: BLE001
            last_err = e
    raise last_err


# revision 37
# speedup vs baseline: 2.3151x; 1.1269x over previous
"""CRD loss kernel for Trainium2, 8-core data-parallel SPMD.

loss = -sum_i( (zs_i . zt_i) / (|zs_i| |zt_i|) ) / B
  zs = f_s @ W_s.T + b_s   [B, 128]
  zt = f_t @ W_t.T + b_t   [B, 128]

Sharding: batch B=16384 split across 8 cores (2048 rows each); projection
weights replicated. Each core emits per-row-chunk partial sums plus the last
block's raw [st|ss|tt] sums; the host folds and scales them.

Per-core dataflow (v10):
  - Host passes x TRANSPOSED ([D, rows], layout prep only) so no PE
    transposes are needed: z.T [feat, rows] = sum_k (W.T chunk).T @ xT chunk
    accumulated straight out of DMA'd xT tiles (f16 x, f16 W, fp32 PSUM).
  - Three DMA queues run in parallel (SP / ACT HWDGE, Pool SWDGE). The pool
    queue cast-stages most x chunk ranges f32->f16 into DRAM scratch per row
    block; SP and ACT then stream the staged f16 (half the bytes of f32),
    while the pool also cast-loads a few chunks directly to SBUF. This keeps
    every DMA queue under the PE matmul floor.
  - W is cast f32->f16 on-device by the pool queue (s part before block 0's
    s tiles, t part after them, so the first matmuls start early).
  - Bias add is fused into the PSUM->SBUF eviction (DVE tensor_scalar, f16
    out for 2x DVE products). Row sums land ON PARTITIONS via
    matmul(lhsT=product chunk, rhs=ones [128,1]) -> [rows128, 1] so the
    normalize tail (reciprocal, sqrt, muls, reduce) is partition-parallel.
  - Each block's row-sum matmuls + tail are deferred until after the NEXT
    block's projection matmuls, so PE never stalls on the DVE chain; the
    last block ships raw sums (host folds them) to shorten the end chain.
  - Tapered first/last blocks (256,512,512,512,256) speed pipeline fill and
    shorten the critical chain behind the last DMA.
"""
import numpy as np

import concourse.bass as bass
import concourse.mybir as mybir
from concourse.tile import TileContext
from concourse import bass_utils

# Problem shapes (hardcoded per contest contract)
B = 16384
DS = 768
DT = 1024
F = 128
NCORES = 8
R = B // NCORES          # rows per core = 2048
NCH_S = DS // 128        # 6
NCH_T = DT // 128        # 8
BLOCKS = [(0, 256), (256, 512), (768, 512), (1280, 512), (1792, 256)]
NBLK = len(BLOCKS)
P = 128

f32 = mybir.dt.float32
f32r = mybir.dt.float32r
f16 = mybir.dt.float16

# chunk -> queue assignment. On middle (512-row) blocks the pool cast-loads
# s0/t0/t7 directly and cast-stages s1..5 / t1..6 to DRAM f16 for sp/act.
# The first and last (256-row) blocks skip staging entirely: sp/act load
# their chunks as f32r directly (with small f32r stationaries), so the
# pipeline fills immediately and the tail chain is short.
S_POOL_N = 1             # s-chunks loaded by pool directly (s0)
T_POOL_N = 1             # t-chunks loaded by pool directly (t0; plus t7)
S_STAGE = NCH_S - S_POOL_N           # staged s-chunks (s1..5)
T_STAGE = NCH_T - 1 - T_POOL_N       # staged t-chunks (t1..6)

# last block ships its raw [st|ss|tt] row-chunk sums (host folds them)
LAST_NCH = BLOCKS[-1][1] // P
OUT_COLS = (NBLK - 1) + 3 * LAST_NCH

_CACHE = {}


def legalize_waits(nc, max_waits=1):
    """Walrus codegen in this container rejects >1 sync-wait per instruction.
    Split extra waits onto same-engine NoOps placed right before the instr."""
    n_fixed = 0
    for fn in nc.m.functions:
        for blk in fn.blocks:
            new_insts = []
            for inst in blk.instructions:
                si = inst.sync_info
                if (
                    si is not None
                    and len(si.on_wait) > max_waits
                    and not isinstance(inst, mybir.InstISA)
                ):
                    waits = list(si.on_wait)
                    extra, keep = waits[:-max_waits], waits[-max_waits:]
                    for j, w in enumerate(extra):
                        nop = mybir.InstNoOp(
                            name=f"{inst.name}-wn{j}", engine=inst.engine
                        )
                        nop.sync_info = mybir.SyncInfo(on_wait=[w], on_update=[])
                        new_insts.append(nop)
                    inst.sync_info = mybir.SyncInfo(
                        on_wait=keep, on_update=list(si.on_update)
                    )
                    n_fixed += 1
                new_insts.append(inst)
            blk.instructions = new_insts
    return n_fixed


def build(repeat=1):
    nc = bass.Bass("TRN2")
    # x transposed on host (layout only): [D, R]; f32r == f32 bits
    fsT = nc.dram_tensor("fsT", [DS, R], f32r, kind="ExternalInput")
    ftT = nc.dram_tensor("ftT", [DT, R], f32r, kind="ExternalInput")
    # W in transposed-chunk layout: wst[p, k*128+f] = W[f, k*128+p]
    wst = nc.dram_tensor("wst", [P, DS + DT], f32r, kind="ExternalInput")
    bst = nc.dram_tensor("bst", [P, 2], f32, kind="ExternalInput")
    out = nc.dram_tensor("out", [P, OUT_COLS], f32, kind="ExternalOutput")
    # f16 staging scratch for the HWDGE-streamed chunk ranges
    fs16 = nc.dram_tensor("fs16", [S_STAGE * P, R], f16, kind="Internal")
    ft16 = nc.dram_tensor("ft16", [T_STAGE * P, R], f16, kind="Internal")

    def chunks3(dram, k0, nch, r0, rows):
        """[nch*128, rows] DRAM slice viewed as [128, nch, rows]."""
        return dram[k0 * P:(k0 + nch) * P, r0:r0 + rows].rearrange(
            "(k p) r -> p k r", p=P
        )

    with TileContext(nc) as tc:
        with (
            tc.tile_pool(name="const", bufs=1) as const,
            tc.tile_pool(name="xs_po", bufs=3) as xs_po_p,
            tc.tile_pool(name="xt_po", bufs=3) as xt_po_p,
            tc.tile_pool(name="xs_sp", bufs=3) as xs_sp_p,
            tc.tile_pool(name="xt_ac", bufs=3) as xt_ac_p,
            tc.tile_pool(name="xt_sp7", bufs=3) as xt_sp7_p,
            tc.tile_pool(name="zprod", bufs=4) as zprod_pool,
            tc.tile_pool(name="tail", bufs=2) as tail_pool,
            tc.tile_pool(name="psum_zs", bufs=2, space="PSUM") as psum_zs_pool,
            tc.tile_pool(name="psum_zt", bufs=2, space="PSUM") as psum_zt_pool,
            tc.tile_pool(name="psum_sum", bufs=2, space="PSUM") as psum_sum_pool,
        ):
            # ---- constants / weights ----
            ones_col = const.tile([P, 1], f16)
            nc.vector.memset(ones_col, 1.0)

            # f16 stationary for ALL chunks, cast on-device by the pool
            # queue; s part first so block 0's s matmuls start early
            wst_f16 = const.tile([P, DS + DT], f16)
            nc.gpsimd.dma_start(wst_f16[:, 0:DS], wst[:, 0:DS])

            # f32r stationary for the first/last blocks' direct loads
            ws_f32r = const.tile([P, S_STAGE * P], f32r)   # s1..5
            nc.sync.dma_start(ws_f32r, wst[:, P:NCH_S * P])
            wt_f32r = const.tile([P, T_STAGE * P], f32r)   # t1..6
            nc.scalar.dma_start(
                wt_f32r, wst[:, (NCH_S + 1) * P:(NCH_S + 7) * P]
            )

            bst_sb = const.tile([P, 2], f32)
            nc.sync.dma_start(bst_sb, bst[:, :])
            bs_col = bst_sb[:, 0:1]
            bt_col = bst_sb[:, 1:2]

            partials = const.tile([P, OUT_COLS], f32)

            def w16(kglob):
                return wst_f16[:, kglob * P:(kglob + 1) * P]

            # ---- main loop over row blocks ----
            # Each block's row-sum matmuls + normalize tail are DEFERRED
            # until after the NEXT block's projection matmuls, so PE's main
            # stream never stalls on the DVE eviction/product chain.
            pending = [None]
            for blk, (r0, rows) in [
                bl for _ in range(repeat) for bl in enumerate(BLOCKS)
            ]:
                last = blk == NBLK - 1
                direct = rows == 256    # first/last block: no staging
                if not direct:
                    # pool: cast-stage the HWDGE chunk ranges to DRAM f16
                    # first (sp/act depend on them)
                    nc.gpsimd.dma_start(
                        fs16[:, r0:r0 + rows],
                        fsT[S_POOL_N * P:NCH_S * P, r0:r0 + rows],
                    )
                    nc.gpsimd.dma_start(
                        ft16[:, r0:r0 + rows],
                        ftT[T_POOL_N * P:(NCH_T - 1) * P, r0:r0 + rows],
                    )
                xs_po = xs_po_p.tile([P, S_POOL_N, rows], f16, tag="xs_po")
                nc.gpsimd.dma_start(xs_po, chunks3(fsT, 0, S_POOL_N, r0, rows))
                if blk == 0:
                    # t part of the f16 weights
                    nc.gpsimd.dma_start(wst_f16[:, DS:], wst[:, DS:])
                xt_po = xt_po_p.tile([P, T_POOL_N, rows], f16, tag="xt_po")
                nc.gpsimd.dma_start(xt_po, chunks3(ftT, 0, T_POOL_N, r0, rows))
                xt_sp7 = xt_sp7_p.tile([P, rows], f16, tag="xt_sp7")
                nc.gpsimd.dma_start(
                    xt_sp7, ftT[(NCH_T - 1) * P:NCH_T * P, r0:r0 + rows]
                )

                if direct:
                    # sp/act pull their chunks straight from the f32 source
                    xs_sp = xs_sp_p.tile(
                        [P, S_STAGE, rows], f32r, tag="xs_sp32"
                    )
                    nc.sync.dma_start(
                        xs_sp, chunks3(fsT, S_POOL_N, S_STAGE, r0, rows)
                    )
                    xt_ac = xt_ac_p.tile(
                        [P, T_STAGE, rows], f32r, tag="xt_ac32"
                    )
                    nc.scalar.dma_start(
                        xt_ac, chunks3(ftT, T_POOL_N, T_STAGE, r0, rows)
                    )
                else:
                    # sp: staged s-chunks; act: staged t-chunks (f16)
                    xs_sp = xs_sp_p.tile([P, S_STAGE, rows], f16, tag="xs_sp")
                    nc.sync.dma_start(
                        xs_sp, chunks3(fs16, 0, S_STAGE, r0, rows)
                    )
                    xt_ac = xt_ac_p.tile([P, T_STAGE, rows], f16, tag="xt_ac")
                    nc.scalar.dma_start(
                        xt_ac, chunks3(ft16, 0, T_STAGE, r0, rows)
                    )

                # ---- projections: z.T [feat, rows] accumulated in PSUM ----
                psz_s = psum_zs_pool.tile([P, rows], f32, tag="psz_s")
                psz_t = psum_zt_pool.tile([P, rows], f32, tag="psz_t")

                def mm_s(psz_s=psz_s, xs_po=xs_po, xs_sp=xs_sp,
                         direct=direct):
                    for k in range(NCH_S):
                        if k < S_POOL_N:
                            src, w = xs_po[:, k, :], w16(k)
                        else:
                            src = xs_sp[:, k - S_POOL_N, :]
                            w = (
                                ws_f32r[:, (k - 1) * P:k * P] if direct
                                else w16(k)
                            )
                        nc.tensor.matmul(
                            psz_s, w, src,
                            start=(k == 0), stop=(k == NCH_S - 1),
                        )

                def mm_t(psz_t=psz_t, xt_po=xt_po, xt_ac=xt_ac,
                         xt_sp7=xt_sp7, direct=direct):
                    for k in range(NCH_T):
                        if k < T_POOL_N:
                            src, w = xt_po[:, k, :], w16(NCH_S + k)
                        elif k < NCH_T - 1:
                            src = xt_ac[:, k - T_POOL_N, :]
                            w = (
                                wt_f32r[:, (k - 1) * P:k * P] if direct
                                else w16(NCH_S + k)
                            )
                        else:
                            src, w = xt_sp7, w16(NCH_S + k)
                        nc.tensor.matmul(
                            psz_t, w, src,
                            start=(k == 0), stop=(k == NCH_T - 1),
                        )

                # last block: t first so the final post-DMA chain is the
                # shorter s branch
                if last:
                    mm_t(); mm_s()
                else:
                    mm_s(); mm_t()

                # previous block's row sums + tail go here, AFTER this
                # block's projection matmuls in PE program order
                if pending[0] is not None:
                    pending[0]()

                # ---- bias + eviction to f16 SBUF (DVE) ----
                zs_sb = zprod_pool.tile([P, rows], f16, tag="zsb")
                zt_sb = zprod_pool.tile([P, rows], f16, tag="ztb")
                nc.vector.tensor_scalar(
                    zs_sb, psz_s, bs_col, None, mybir.AluOpType.add
                )
                nc.vector.tensor_scalar(
                    zt_sb, psz_t, bt_col, None, mybir.AluOpType.add
                )

                # products and squares (f16, 2x DVE throughput)
                prod_st = zprod_pool.tile([P, rows], f16, tag="prod")
                zs2 = zprod_pool.tile([P, rows], f16, tag="zs2")
                zt2 = zprod_pool.tile([P, rows], f16, tag="zt2")
                nc.vector.tensor_mul(prod_st, zs_sb, zt_sb)
                nc.vector.tensor_mul(zs2, zs_sb, zs_sb)
                nc.vector.tensor_mul(zt2, zt_sb, zt_sb)

                def flush(blk=blk, rows=rows, prod_st=prod_st, zs2=zs2,
                          zt2=zt2, last=last):
                    # row sums on PARTITIONS: matmul(lhsT=product chunk
                    # [feat, rows128], rhs=ones [feat,1]) -> [rows128, 1].
                    # sumsT columns: c + nchunks*{0: st, 1: ss, 2: tt}.
                    nchunks = rows // P
                    sumsT = psum_sum_pool.tile(
                        [P, 3 * nchunks], f32, tag="sumsT"
                    )
                    for i, src in enumerate((prod_st, zs2, zt2)):
                        for c in range(nchunks):
                            nc.tensor.matmul(
                                sumsT[:, i * nchunks + c:i * nchunks + c + 1],
                                src[:, c * P:(c + 1) * P],
                                ones_col,
                                start=True,
                                stop=True,
                            )
                    if last:
                        # ship raw [st|ss|tt] sums; host folds the normalize
                        nc.vector.tensor_copy(
                            partials[:, NBLK - 1:NBLK - 1 + 3 * nchunks],
                            sumsT,
                        )
                        return
                    # normalize tail, partition-parallel:
                    # partial = sum st * rsqrt(ss) * rsqrt(tt)
                    q = tail_pool.tile([P, 2 * nchunks], f32, tag="q")
                    nc.vector.reciprocal(q, sumsT[:, nchunks:3 * nchunks])
                    q2 = tail_pool.tile([P, 2 * nchunks], f32, tag="q2")
                    nc.scalar.activation(
                        q2, q, mybir.ActivationFunctionType.Sqrt
                    )
                    v = tail_pool.tile([P, nchunks], f32, tag="v")
                    nc.vector.tensor_mul(
                        v, q2[:, 0:nchunks], q2[:, nchunks:2 * nchunks]
                    )
                    w_ = tail_pool.tile([P, nchunks], f32, tag="w")
                    nc.vector.tensor_mul(w_, sumsT[:, 0:nchunks], v)
                    nc.vector.reduce_sum(
                        partials[:, blk:blk + 1], w_, axis=mybir.AxisListType.X
                    )

                pending[0] = flush
                if last:
                    # ship all but the last block's columns early so only the
                    # final raw sums are on the critical tail
                    nc.sync.dma_start(
                        out[:, 0:NBLK - 1], partials[:, 0:NBLK - 1]
                    )

            pending[0]()
            # last block's raw sums; host does the final normalize + sum
            nc.sync.dma_start(
                out[:, NBLK - 1:OUT_COLS], partials[:, NBLK - 1:OUT_COLS]
            )

    legalize_waits(nc)
    return nc


def get_nc():
    if "nc" not in _CACHE:
        _CACHE["nc"] = build()
    return _CACHE["nc"]


def make_in_maps(f_s, f_t, W_s, b_s, W_t, b_t):
    f_s = np.ascontiguousarray(np.asarray(f_s, dtype=np.float32))
    f_t = np.ascontiguousarray(np.asarray(f_t, dtype=np.float32))
    W_s = np.asarray(W_s, dtype=np.float32)
    b_s = np.asarray(b_s, dtype=np.float32)
    W_t = np.asarray(W_t, dtype=np.float32)
    b_t = np.asarray(b_t, dtype=np.float32)

    # layout prep (no arithmetic): x transposed, W in transposed-chunk form
    fsT = np.ascontiguousarray(f_s.T)    # [DS, B]
    ftT = np.ascontiguousarray(f_t.T)    # [DT, B]
    wst = np.empty((P, DS + DT), dtype=np.float32)
    for k in range(NCH_S):
        wst[:, k * P:(k + 1) * P] = W_s[:, k * P:(k + 1) * P].T
    for k in range(NCH_T):
        wst[:, (NCH_S + k) * P:(NCH_S + k + 1) * P] = W_t[:, k * P:(k + 1) * P].T
    bst = np.ascontiguousarray(np.stack([b_s, b_t], axis=1))  # [128, 2]

    in_maps = []
    for c in range(NCORES):
        sl = slice(c * R, (c + 1) * R)
        in_maps.append(
            {
                "fsT": np.ascontiguousarray(fsT[:, sl]),
                "ftT": np.ascontiguousarray(ftT[:, sl]),
                "wst": wst,
                "bst": bst,
            }
        )
    return in_maps


def combine(results):
    total = 0.0
    for c in range(NCORES):
        o = results[c]["out"].astype(np.float64)
        total += o[:, 0:NBLK - 1].sum()
        # last block shipped raw sums: st / sqrt(ss * tt) per row chunk
        st = o[:, NBLK - 1:NBLK - 1 + LAST_NCH]
        ss = o[:, NBLK - 1 + LAST_NCH:NBLK - 1 + 2 * LAST_NCH]
        tt = o[:, NBLK - 1 + 2 * LAST_NCH:NBLK - 1 + 3 * LAST_NCH]
        total += (st / np.sqrt(ss * tt)).sum()
    loss = -(total / B)
    return np.array([loss], dtype=np.float32)


def kernel(f_s, f_t, W_s, b_s, W_t, b_t):
    nc = get_nc()
    in_maps = make_in_maps(f_s, f_t, W_s, b_s, W_t, b_t)
    last_err = None
    for _ in range(3):  # retry transient device wedges (NRT_EXEC_UNIT_...)
        try:
            res = bass_utils.run_bass_kernel_spmd(
                nc, in_maps, core_ids=list(range(NCORES))
            )
            return combine(res.results)
        except Exception as e:  # noqa: BLE001
            last_err = e
    raise last_err


# revision 44
# speedup vs baseline: 2.3638x; 1.0210x over previous
"""CRD loss kernel for Trainium2, 8-core data-parallel SPMD.

loss = -sum_i( (zs_i . zt_i) / (|zs_i| |zt_i|) ) / B
  zs = f_s @ W_s.T + b_s   [B, 128]
  zt = f_t @ W_t.T + b_t   [B, 128]

Sharding: batch B=16384 split across 8 cores (2048 rows each); projection
weights replicated. Each core emits per-row-chunk partial sums plus the last
block's raw [st|ss|tt] sums; the host folds and scales them.

Per-core dataflow (v10):
  - Host passes x TRANSPOSED ([D, rows], layout prep only) so no PE
    transposes are needed: z.T [feat, rows] = sum_k (W.T chunk).T @ xT chunk
    accumulated straight out of DMA'd xT tiles (f16 x, f16 W, fp32 PSUM).
  - Three DMA queues run in parallel (SP / ACT HWDGE, Pool SWDGE). The pool
    queue cast-stages most x chunk ranges f32->f16 into DRAM scratch per row
    block; SP and ACT then stream the staged f16 (half the bytes of f32),
    while the pool also cast-loads a few chunks directly to SBUF. This keeps
    every DMA queue under the PE matmul floor.
  - W is cast f32->f16 on-device by the pool queue (s part before block 0's
    s tiles, t part after them, so the first matmuls start early).
  - Bias add is fused into the PSUM->SBUF eviction (DVE tensor_scalar, f16
    out for 2x DVE products). Row sums land ON PARTITIONS via
    matmul(lhsT=product chunk, rhs=ones [128,1]) -> [rows128, 1] so the
    normalize tail (reciprocal, sqrt, muls, reduce) is partition-parallel.
  - Each block's row-sum matmuls + tail are deferred until after the NEXT
    block's projection matmuls, so PE never stalls on the DVE chain; the
    last block's eviction/copy chain runs on the then-idle ACT engine and
    ships raw sums (host folds them) to shorten the end chain.
"""
import numpy as np

import concourse.bass as bass
import concourse.mybir as mybir
from concourse.tile import TileContext
from concourse import bass_utils

# Problem shapes (hardcoded per contest contract)
B = 16384
DS = 768
DT = 1024
F = 128
NCORES = 8
R = B // NCORES          # rows per core = 2048
NCH_S = DS // 128        # 6
NCH_T = DT // 128        # 8
BLOCKS = [(0, 512), (512, 512), (1024, 512), (1536, 512)]
NBLK = len(BLOCKS)
P = 128

f32 = mybir.dt.float32
f32r = mybir.dt.float32r
f16 = mybir.dt.float16

# chunk -> queue assignment. Pool cast-loads s0/t0/t7 directly and
# cast-stages the rest to DRAM f16; sp streams staged s1..5, act t1..6.
S_POOL_N = 1             # s-chunks loaded by pool directly
T_POOL_N = 1             # t-chunks loaded by pool directly (plus t7)
S_STAGE = NCH_S - S_POOL_N   # staged s-chunks (s1..5)
T_STAGE = NCH_T - T_POOL_N   # staged t-chunks (t1..7); t7 itself is direct

# last block ships its raw [st|ss|tt] row-chunk sums (host folds them)
LAST_NCH = BLOCKS[-1][1] // P
OUT_COLS = (NBLK - 1) + 3 * LAST_NCH

_CACHE = {}


def legalize_waits(nc, max_waits=1):
    """Walrus codegen in this container rejects >1 sync-wait per instruction.
    Split extra waits onto same-engine NoOps placed right before the instr."""
    n_fixed = 0
    for fn in nc.m.functions:
        for blk in fn.blocks:
            new_insts = []
            for inst in blk.instructions:
                si = inst.sync_info
                if (
                    si is not None
                    and len(si.on_wait) > max_waits
                    and not isinstance(inst, mybir.InstISA)
                ):
                    waits = list(si.on_wait)
                    extra, keep = waits[:-max_waits], waits[-max_waits:]
                    for j, w in enumerate(extra):
                        nop = mybir.InstNoOp(
                            name=f"{inst.name}-wn{j}", engine=inst.engine
                        )
                        nop.sync_info = mybir.SyncInfo(on_wait=[w], on_update=[])
                        new_insts.append(nop)
                    inst.sync_info = mybir.SyncInfo(
                        on_wait=keep, on_update=list(si.on_update)
                    )
                    n_fixed += 1
                new_insts.append(inst)
            blk.instructions = new_insts
    return n_fixed


def build(repeat=1):
    nc = bass.Bass("TRN2")
    # x transposed on host (layout only): [D, R]; f32r == f32 bits
    fsT = nc.dram_tensor("fsT", [DS, R], f32r, kind="ExternalInput")
    ftT = nc.dram_tensor("ftT", [DT, R], f32r, kind="ExternalInput")
    # W in transposed-chunk layout: wst[p, k*128+f] = W[f, k*128+p]
    wst = nc.dram_tensor("wst", [P, DS + DT], f32r, kind="ExternalInput")
    bst = nc.dram_tensor("bst", [P, 2], f32, kind="ExternalInput")
    out = nc.dram_tensor("out", [P, OUT_COLS], f32, kind="ExternalOutput")
    # f16 staging scratch for the HWDGE-streamed chunk ranges
    fs16 = nc.dram_tensor("fs16", [S_STAGE * P, R], f16, kind="Internal")
    ft16 = nc.dram_tensor("ft16", [(T_STAGE - 1) * P, R], f16, kind="Internal")

    def chunks3(dram, k0, nch, r0, rows):
        """[nch*128, rows] DRAM slice viewed as [128, nch, rows]."""
        return dram[k0 * P:(k0 + nch) * P, r0:r0 + rows].rearrange(
            "(k p) r -> p k r", p=P
        )

    with TileContext(nc) as tc:
        with (
            tc.tile_pool(name="const", bufs=1) as const,
            tc.tile_pool(name="xs_po", bufs=3) as xs_po_p,
            tc.tile_pool(name="xt_po", bufs=3) as xt_po_p,
            tc.tile_pool(name="xs_sp", bufs=3) as xs_sp_p,
            tc.tile_pool(name="xt_ac", bufs=3) as xt_ac_p,
            tc.tile_pool(name="xt_sp7", bufs=3) as xt_sp7_p,
            tc.tile_pool(name="zprod", bufs=4) as zprod_pool,
            tc.tile_pool(name="tail", bufs=2) as tail_pool,
            tc.tile_pool(name="psum_zs", bufs=2, space="PSUM") as psum_zs_pool,
            tc.tile_pool(name="psum_zt", bufs=2, space="PSUM") as psum_zt_pool,
            tc.tile_pool(name="psum_sum", bufs=2, space="PSUM") as psum_sum_pool,
        ):
            # ---- constants / weights ----
            ones_col = const.tile([P, 1], f16)
            nc.vector.memset(ones_col, 1.0)

            # f16 stationary for ALL chunks, cast on-device by the pool
            # queue; s part first so block 0's s matmuls start early
            wst_f16 = const.tile([P, DS + DT], f16)
            nc.gpsimd.dma_start(wst_f16[:, 0:DS], wst[:, 0:DS])

            bst_sb = const.tile([P, 2], f32)
            nc.sync.dma_start(bst_sb, bst[:, :])
            bs_col = bst_sb[:, 0:1]
            bt_col = bst_sb[:, 1:2]

            partials = const.tile([P, OUT_COLS], f32)

            def w16(kglob):
                return wst_f16[:, kglob * P:(kglob + 1) * P]

            # ---- main loop over row blocks ----
            # Each block's row-sum matmuls + normalize tail are DEFERRED
            # until after the NEXT block's projection matmuls, so PE's main
            # stream never stalls on the DVE eviction/product chain.
            pending = [None]
            for blk, (r0, rows) in [
                bl for _ in range(repeat) for bl in enumerate(BLOCKS)
            ]:
                last = blk == NBLK - 1
                # pool: cast-stage the HWDGE chunk ranges to DRAM f16 first
                # (sp/act depend on them), then cast-load its own chunks
                nc.gpsimd.dma_start(
                    fs16[:, r0:r0 + rows],
                    fsT[S_POOL_N * P:NCH_S * P, r0:r0 + rows],
                )
                nc.gpsimd.dma_start(
                    ft16[:, r0:r0 + rows],
                    ftT[T_POOL_N * P:(NCH_T - 1) * P, r0:r0 + rows],
                )
                xs_po = xs_po_p.tile([P, S_POOL_N, rows], f16, tag="xs_po")
                nc.gpsimd.dma_start(xs_po, chunks3(fsT, 0, S_POOL_N, r0, rows))
                if blk == 0:
                    # t part of the f16 weights
                    nc.gpsimd.dma_start(wst_f16[:, DS:], wst[:, DS:])
                xt_po = xt_po_p.tile([P, T_POOL_N, rows], f16, tag="xt_po")
                nc.gpsimd.dma_start(xt_po, chunks3(ftT, 0, T_POOL_N, r0, rows))
                xt_sp7 = xt_sp7_p.tile([P, rows], f16, tag="xt_sp7")
                nc.gpsimd.dma_start(
                    xt_sp7, ftT[(NCH_T - 1) * P:NCH_T * P, r0:r0 + rows]
                )

                # sp: staged s-chunks; act: staged t-chunks. Block 0's
                # loads are split so the pipeline fills sooner.
                xs_sp = xs_sp_p.tile([P, S_STAGE, rows], f16, tag="xs_sp")
                xt_ac = xt_ac_p.tile([P, T_STAGE - 1, rows], f16, tag="xt_ac")
                if blk == 0:
                    h = S_STAGE // 2
                    nc.sync.dma_start(
                        xs_sp[:, 0:h, :], chunks3(fs16, 0, h, r0, rows)
                    )
                    nc.sync.dma_start(
                        xs_sp[:, h:, :],
                        chunks3(fs16, h, S_STAGE - h, r0, rows),
                    )
                    g = (T_STAGE - 1) // 2
                    nc.scalar.dma_start(
                        xt_ac[:, 0:g, :], chunks3(ft16, 0, g, r0, rows)
                    )
                    nc.scalar.dma_start(
                        xt_ac[:, g:, :],
                        chunks3(ft16, g, T_STAGE - 1 - g, r0, rows),
                    )
                else:
                    nc.sync.dma_start(
                        xs_sp, chunks3(fs16, 0, S_STAGE, r0, rows)
                    )
                    nc.scalar.dma_start(
                        xt_ac, chunks3(ft16, 0, T_STAGE - 1, r0, rows)
                    )

                # ---- projections: z.T [feat, rows] accumulated in PSUM ----
                psz_s = psum_zs_pool.tile([P, rows], f32, tag="psz_s")
                psz_t = psum_zt_pool.tile([P, rows], f32, tag="psz_t")

                def mm_s(psz_s=psz_s, xs_po=xs_po, xs_sp=xs_sp):
                    for k in range(NCH_S):
                        src = (
                            xs_po[:, k, :] if k < S_POOL_N
                            else xs_sp[:, k - S_POOL_N, :]
                        )
                        nc.tensor.matmul(
                            psz_s, w16(k), src,
                            start=(k == 0), stop=(k == NCH_S - 1),
                        )

                def mm_t(psz_t=psz_t, xt_po=xt_po, xt_ac=xt_ac,
                         xt_sp7=xt_sp7):
                    for k in range(NCH_T):
                        if k < T_POOL_N:
                            src = xt_po[:, k, :]
                        elif k < NCH_T - 1:
                            src = xt_ac[:, k - T_POOL_N, :]
                        else:
                            src = xt_sp7
                        nc.tensor.matmul(
                            psz_t, w16(NCH_S + k), src,
                            start=(k == 0), stop=(k == NCH_T - 1),
                        )

                # last block: t first so the final post-DMA chain is the
                # shorter s branch. Block 0: pool-direct chunks of both
                # branches first so PE has work while staged tiles land.
                if last:
                    mm_t(); mm_s()
                elif blk == 0:
                    for k in range(S_POOL_N):
                        nc.tensor.matmul(
                            psz_s, w16(k), xs_po[:, k, :],
                            start=(k == 0), stop=False,
                        )
                    for k in range(T_POOL_N):
                        nc.tensor.matmul(
                            psz_t, w16(NCH_S + k), xt_po[:, k, :],
                            start=(k == 0), stop=False,
                        )
                    nc.tensor.matmul(
                        psz_t, w16(NCH_S + NCH_T - 1), xt_sp7,
                        start=False, stop=False,
                    )
                    for k in range(S_POOL_N, NCH_S):
                        nc.tensor.matmul(
                            psz_s, w16(k), xs_sp[:, k - S_POOL_N, :],
                            start=False, stop=(k == NCH_S - 1),
                        )
                    for k in range(T_POOL_N, NCH_T - 1):
                        nc.tensor.matmul(
                            psz_t, w16(NCH_S + k), xt_ac[:, k - T_POOL_N, :],
                            start=False, stop=(k == NCH_T - 2),
                        )
                else:
                    mm_s(); mm_t()

                # previous block's row sums + tail go here, AFTER this
                # block's projection matmuls in PE program order
                if pending[0] is not None:
                    pending[0]()

                # ---- bias + eviction to f16 SBUF ----
                # Steady state: DVE (ACT is on DMA duty). Last block: ACT,
                # which is idle by then, so the final chain skips the DVE
                # queue backlog.
                zs_sb = zprod_pool.tile([P, rows], f16, tag="zsb")
                zt_sb = zprod_pool.tile([P, rows], f16, tag="ztb")
                prod_st = zprod_pool.tile([P, rows], f16, tag="prod")
                zs2 = zprod_pool.tile([P, rows], f16, tag="zs2")
                zt2 = zprod_pool.tile([P, rows], f16, tag="zt2")
                if last:
                    # t branch stopped first: zt evicts on DVE while the s
                    # matmuls finish; zs evicts on the idle ACT in parallel
                    nc.vector.tensor_scalar(
                        zt_sb, psz_t, bt_col, None, mybir.AluOpType.add
                    )
                    nc.vector.tensor_mul(zt2, zt_sb, zt_sb)
                    nc.scalar.activation(
                        zs_sb, psz_s, mybir.ActivationFunctionType.Identity,
                        bias=bs_col,
                    )
                    nc.vector.tensor_mul(prod_st, zs_sb, zt_sb)
                    nc.scalar.square(zs2, zs_sb)
                else:
                    nc.vector.tensor_scalar(
                        zs_sb, psz_s, bs_col, None, mybir.AluOpType.add
                    )
                    nc.vector.tensor_scalar(
                        zt_sb, psz_t, bt_col, None, mybir.AluOpType.add
                    )
                    nc.vector.tensor_mul(prod_st, zs_sb, zt_sb)
                    nc.vector.tensor_mul(zs2, zs_sb, zs_sb)
                    nc.vector.tensor_mul(zt2, zt_sb, zt_sb)

                def flush(blk=blk, rows=rows, prod_st=prod_st, zs2=zs2,
                          zt2=zt2, last=last):
                    # row sums on PARTITIONS: matmul(lhsT=product chunk
                    # [feat, rows128], rhs=ones [feat,1]) -> [rows128, 1].
                    # sumsT columns: c + nchunks*{0: st, 1: ss, 2: tt}.
                    nchunks = rows // P
                    sumsT = psum_sum_pool.tile(
                        [P, 3 * nchunks], f32, tag="sumsT"
                    )
                    for i, src in enumerate((prod_st, zs2, zt2)):
                        for c in range(nchunks):
                            nc.tensor.matmul(
                                sumsT[:, i * nchunks + c:i * nchunks + c + 1],
                                src[:, c * P:(c + 1) * P],
                                ones_col,
                                start=True,
                                stop=True,
                            )
                    if last:
                        # ship raw [st|ss|tt] sums via ACT (idle by now);
                        # host folds the normalize
                        nc.scalar.copy(
                            partials[:, NBLK - 1:NBLK - 1 + 3 * nchunks],
                            sumsT,
                        )
                        return
                    # normalize tail, partition-parallel:
                    # partial = sum st * rsqrt(ss) * rsqrt(tt)
                    q = tail_pool.tile([P, 2 * nchunks], f32, tag="q")
                    nc.vector.reciprocal(q, sumsT[:, nchunks:3 * nchunks])
                    q2 = tail_pool.tile([P, 2 * nchunks], f32, tag="q2")
                    nc.scalar.activation(
                        q2, q, mybir.ActivationFunctionType.Sqrt
                    )
                    v = tail_pool.tile([P, nchunks], f32, tag="v")
                    nc.vector.tensor_mul(
                        v, q2[:, 0:nchunks], q2[:, nchunks:2 * nchunks]
                    )
                    w_ = tail_pool.tile([P, nchunks], f32, tag="w")
                    nc.vector.tensor_mul(w_, sumsT[:, 0:nchunks], v)
                    nc.vector.reduce_sum(
                        partials[:, blk:blk + 1], w_, axis=mybir.AxisListType.X
                    )

                pending[0] = flush
                if last:
                    # ship all but the last block's columns early so only the
                    # final raw sums are on the critical tail
                    nc.sync.dma_start(
                        out[:, 0:NBLK - 1], partials[:, 0:NBLK - 1]
                    )

            pending[0]()
            # last block's raw sums, issued from ACT right behind its copy;
            # host does the final normalize + sum
            nc.scalar.dma_start(
                out[:, NBLK - 1:OUT_COLS], partials[:, NBLK - 1:OUT_COLS]
            )

    legalize_waits(nc)
    return nc


def get_nc():
    if "nc" not in _CACHE:
        _CACHE["nc"] = build()
    return _CACHE["nc"]


def make_in_maps(f_s, f_t, W_s, b_s, W_t, b_t):
    f_s = np.ascontiguousarray(np.asarray(f_s, dtype=np.float32))
    f_t = np.ascontiguousarray(np.asarray(f_t, dtype=np.float32))
    W_s = np.asarray(W_s, dtype=np.float32)
    b_s = np.asarray(b_s, dtype=np.float32)
    W_t = np.asarray(W_t, dtype=np.float32)
    b_t = np.asarray(b_t, dtype=np.float32)

    # layout prep (no arithmetic): x transposed, W in transposed-chunk form
    fsT = np.ascontiguousarray(f_s.T)    # [DS, B]
    ftT = np.ascontiguousarray(f_t.T)    # [DT, B]
    wst = np.empty((P, DS + DT), dtype=np.float32)
    for k in range(NCH_S):
        wst[:, k * P:(k + 1) * P] = W_s[:, k * P:(k + 1) * P].T
    for k in range(NCH_T):
        wst[:, (NCH_S + k) * P:(NCH_S + k + 1) * P] = W_t[:, k * P:(k + 1) * P].T
    bst = np.ascontiguousarray(np.stack([b_s, b_t], axis=1))  # [128, 2]

    in_maps = []
    for c in range(NCORES):
        sl = slice(c * R, (c + 1) * R)
        in_maps.append(
            {
                "fsT": np.ascontiguousarray(fsT[:, sl]),
                "ftT": np.ascontiguousarray(ftT[:, sl]),
                "wst": wst,
                "bst": bst,
            }
        )
    return in_maps


def combine(results):
    total = 0.0
    for c in range(NCORES):
        o = results[c]["out"].astype(np.float64)
        total += o[:, 0:NBLK - 1].sum()
        # last block shipped raw sums: st / sqrt(ss * tt) per row chunk
        st = o[:, NBLK - 1:NBLK - 1 + LAST_NCH]
        ss = o[:, NBLK - 1 + LAST_NCH:NBLK - 1 + 2 * LAST_NCH]
        tt = o[:, NBLK - 1 + 2 * LAST_NCH:NBLK - 1 + 3 * LAST_NCH]
        total += (st / np.sqrt(ss * tt)).sum()
    loss = -(total / B)
    return np.array([loss], dtype=np.float32)


def kernel(f_s, f_t, W_s, b_s, W_t, b_t):
    nc = get_nc()
    in_maps = make_in_maps(f_s, f_t, W_s, b_s, W_t, b_t)
    last_err = None
    for _ in range(3):  # retry transient device wedges (NRT_EXEC_UNIT_...)
        try:
            res = bass_utils.run_bass_kernel_spmd(
                nc, in_maps, core_ids=list(range(NCORES))
            )
            return combine(res.results)
        except Exception as e:  # noqa: BLE001
            last_err = e
    raise last_err
